# revision 30
# baseline (speedup 1.0000x reference)
"""Trainium2 Bass kernel for nn_Canny_61100204753382 (8-core SPMD).

Sharding: spatial row-bands (64 output rows x all 8 images per core). The
reference's flat-gather quirk reads all_filtered[k_pos, b, i, j] - the
direction index lands in the batch slot and the pixel's own batch index
selects the direction offset - so the coupling between images is at the SAME
pixel position and row-band sharding stays core-local given a small row halo.

Pipeline per column-chunk (CW=117 output cols from 128 input cols, +1
extended G column so each chunk owns its right-neighbor boundary):
  stepA: vertical 11-tap banded matmuls in bf16 hi/lo 3-pass (error ~2^-17);
  stepB: horizontal banded matmuls in f32, 2 images per call; per-channel
  squares evacuated via ACT, magnitude summed on Pool; channel-summed
  gxs/gys (f32) for sector masks; NMS with 4 direct direction compares and
  4 derived via NOT(shifted C) (exact modulo float ties, masked by the
  thresholds); hysteresis via row/col sums; conv(w) -> nms(w) -> fin(w-1)
  interleave keeps PE/DVE/ACT/Pool all busy.
"""

import math
import numpy as np
from contextlib import ExitStack

import concourse.bass as bass
import concourse.mybir as mybir
import concourse.tile as tile
from concourse.bass_utils import run_bass_kernel_spmd
from concourse.alu_op_type import AluOpType

f32 = mybir.dt.float32
f32r = mybir.dt.float32r
bf16 = mybir.dt.bfloat16
u8 = mybir.dt.uint8
AF = mybir.ActivationFunctionType

B, C, H, W = 8, 3, 512, 512
NCORES = 8
RB = H // NCORES          # output rows per core
XR = RB + 14              # input rows per core (7-row halo each side)
XC = W + 14               # padded cols
GR = RB + 4               # G rows per band (final rows -2..65)
CW = 117                  # chunk stride (128 in-cols -> 117 out-cols + 1 ext)
NW = -(-(W + 4) // CW)    # 5 column chunks
PC = CW                   # partitions used by NMS ops
PE1 = CW + 1              # extended G columns per chunk
WIN = RB + 2              # is_max row window (final rows -1..64)
T1 = float(math.tan(math.pi / 8))
T2 = float(math.tan(3 * math.pi / 8))
LOW, HIGH = 0.1, 0.3
NEIGH = [(0, 1), (1, 1), (1, 0), (1, -1), (0, -1), (-1, -1), (-1, 0), (-1, 1)]

_CACHE = {}
TRACE = False
LAST_EXEC_NS = None


def _band(comp, K, M, taps=11):
    Wb = np.zeros((K, M), np.float32)
    for k in range(K):
        for m in range(M):
            if 0 <= k - m < taps:
                Wb[k, m] = comp[k - m]
    return Wb


def _chunk_dims(w):
    s = CW * w
    kw = min(128, XC - s)           # in-cols this chunk
    mw = min(CW, (W + 4) - s)       # out (G) cols this chunk
    return s, kw, mw


def _build():
    nc = bass.Bass()
    # x pre-chunked on host: [XR, NW, B*C, 128] (chunk w zero-padded to 128)
    xh_d = nc.dram_tensor("xh", [XR, NW, B * C, 128], bf16, kind="ExternalInput")
    xl_d = nc.dram_tensor("xl", [XR, NW, B * C, 128], bf16, kind="ExternalInput")
    wah_d = nc.dram_tensor("wah", [XR, 2, 68], bf16, kind="ExternalInput")
    wal_d = nc.dram_tensor("wal", [XR, 2, 68], bf16, kind="ExternalInput")
    wb_d = nc.dram_tensor("wb", [128, 2, PE1], f32, kind="ExternalInput")
    o_d = nc.dram_tensor("o", [PC, NW, B, RB], bf16, kind="ExternalOutput")

    with tile.TileContext(nc) as tc, ExitStack() as ctx:
        P = ctx.enter_context
        const = P(tc.tile_pool(name="const", bufs=1))
        big = P(tc.tile_pool(name="big", bufs=1))
        xp = P(tc.tile_pool(name="xp", bufs=2))
        ev = P(tc.tile_pool(name="ev", bufs=2))
        nm = P(tc.tile_pool(name="nm", bufs=1))
        psA = P(tc.tile_pool(name="psA", bufs=3, space="PSUM"))
        psB = P(tc.tile_pool(name="psB", bufs=2, space="PSUM"))
        psS = P(tc.tile_pool(name="psS", bufs=1, space="PSUM"))

        wah_sb = const.tile([XR, 2, 68], bf16, tag="wah_sb")
        nc.sync.dma_start(wah_sb[:], wah_d[:])
        wal_sb = const.tile([XR, 2, 68], bf16, tag="wal_sb")
        nc.sync.dma_start(wal_sb[:], wal_d[:])
        wb_sb = const.tile([128, 2, PE1], f32, tag="wb_sb")
        nc.sync.dma_start(wb_sb[:], wb_d[:])

        G = big.tile([128, NW, B, GR], f32, name="G", tag="G")
        GXS = big.tile([128, NW, B, GR], f32, tag="GXS")
        GYS = big.tile([128, NW, B, GR], f32, tag="GYS")
        Gp1 = big.tile([128, NW, B, GR], f32, tag="Gp1")
        c1m = big.tile([128, NW, B, GR], u8, tag="c1m")
        c2m = big.tile([128, NW, B, GR], u8, tag="c2m")
        qsm = big.tile([128, NW, B, GR], u8, tag="qsm")
        F = big.tile([128, NW, 8, 4, GR], bf16, tag="F")
        him = big.tile([128, NW, B, GR], bf16, tag="him")
        hi = big.tile([128, NW, B, GR], bf16, tag="hi")
        mid = big.tile([128, NW, B, GR], bf16, tag="mid")
        rs2 = big.tile([128, NW, B, RB], bf16, tag="rs2")
        rsc = big.tile([128, NW, B, RB], bf16, tag="rsc")
        cball = big.tile([128, 8, B, GR], bf16, tag="cball")
        ftmp = big.tile([128, 4, 4, GR], bf16, tag="ftmp")
        nc.vector.memset(cball[0:1, 3:6], 0.0)

        def nms_front(w):
            # sector masks (f32 exact, full GR rows for contiguous APs)
            wax = GXS[0:PC, w]
            way = GYS[0:PC, w]
            qpr = ev.tile([128, B, GR], bf16, tag="qpr")
            nc.vector.tensor_tensor(qpr[0:PC], wax, way, AluOpType.mult)
            nc.vector.tensor_single_scalar(qsm[0:PC, w], qpr[0:PC], 0.0,
                                           AluOpType.is_ge)
            nc.scalar.activation(GXS[0:PC, w], GXS[0:PC, w], AF.Abs)
            nc.scalar.activation(GYS[0:PC, w], GYS[0:PC, w], AF.Abs)
            nc.vector.scalar_tensor_tensor(c1m[0:PC, w], wax, T1, way,
                                           AluOpType.mult, AluOpType.is_gt)
            nc.vector.scalar_tensor_tensor(c2m[0:PC, w], wax, T2, way,
                                           AluOpType.mult, AluOpType.is_lt)
            # direct compares b = 0,1,2,7; partners derived as NOT(shifted C)
            # (exact modulo float ties, which the threshold masks zero out)
            nc.vector.tensor_tensor(cball[0:PC, 0], G[0:PC, w],
                                    Gp1[0:PC, w], AluOpType.is_gt)
            nc.vector.tensor_tensor(cball[0:PC, 1, :, 0:67],
                                    G[0:PC, w, :, 0:67],
                                    Gp1[0:PC, w, :, 1:68], AluOpType.is_gt)
            nc.vector.tensor_tensor(cball[0:PC, 2, :, 0:67],
                                    G[0:PC, w, :, 0:67],
                                    G[0:PC, w, :, 1:68], AluOpType.is_gt)
            nc.vector.tensor_tensor(cball[0:PC, 7, :, 1:68],
                                    G[0:PC, w, :, 1:68],
                                    Gp1[0:PC, w, :, 0:67], AluOpType.is_gt)

        def conv_chunk(w):
            s, kw, mw = _chunk_dims(w)
            mwE = min(PE1, (W + 4) - s)
            if w > 0:
                # boundary partition 0 of derived cball slots from the
                # PREVIOUS chunk (before nms_front(w) overwrites the planes)
                nc.sync.dma_start(cball[0:1, 4, :, 1:67],
                                  cball[PC - 1:PC, 0, :, 1:67])
                nc.sync.dma_start(cball[0:1, 5, :, 1:67],
                                  cball[PC - 1:PC, 1, :, 0:66])
                nc.sync.dma_start(cball[0:1, 3, :, 1:67],
                                  cball[PC - 1:PC, 7, :, 2:68])
            xh_sb = xp.tile([XR, B * C, 128], bf16, tag="xh_sb")
            nc.sync.dma_start(xh_sb[:], xh_d[:, w])
            xl_sb = xp.tile([XR, B * C, 128], bf16, tag="xl_sb")
            nc.sync.dma_start(xl_sb[:], xl_d[:, w])
            gxA = ev.tile([128, B, C, 2, 68], f32, tag="gxA")
            for img in range(B):
                pa = psA.tile([128, 3, 2, 68], f32, tag="pa")
                for ci in range(3):
                    lhT = xh_sb[0:XR, img * C + ci, 0:kw]
                    llT = xl_sb[0:XR, img * C + ci, 0:kw]
                    nc.tensor.matmul(pa[0:kw, ci], lhT, wah_sb[0:XR],
                                     start=True, stop=False)
                    nc.tensor.matmul(pa[0:kw, ci], lhT, wal_sb[0:XR],
                                     start=False, stop=False)
                    nc.tensor.matmul(pa[0:kw, ci], llT, wah_sb[0:XR],
                                     start=False, stop=True)
                nc.scalar.copy(gxA[0:kw, img], pa[0:kw])
            gsA = ev.tile([128, B, 2, 68], f32, tag="gsA")
            sq = ev.tile([128, B, 2, 3, GR], f32, tag="sq", bufs=1)
            mag = ev.tile([128, B, 3, GR], f32, tag="mag", bufs=1)
            tg = ev.tile([128, B, GR], f32, tag="tg", bufs=1)
            for img in range(1, B, 2):
                pb = psB.tile([PE1, 2, 512], f32, tag="pb")
                for j in range(2):
                    nc.tensor.matmul(pb[0:mwE, j, 0:408], wb_sb[0:kw, j, 0:mwE],
                                     gxA[0:kw, img - 1:img + 1, :, j],
                                     start=True, stop=True)
                    nc.scalar.square(sq[0:mwE, img - 1:img + 1, j],
                                     pb[0:mwE, j, 0:408])
                if img % 4 == 3:
                    h = img // 4
                    hs = slice(h * 4, h * 4 + 4)
                    nc.gpsimd.tensor_tensor(gsA[:, hs], gxA[:, hs, 0],
                                            gxA[:, hs, 1], AluOpType.add)
                    nc.gpsimd.tensor_tensor(gsA[:, hs], gsA[:, hs],
                                            gxA[:, hs, 2], AluOpType.add)
                    for j in range(2):
                        pS = psS.tile([PC, 4, 68], f32, tag="pS")
                        nc.tensor.matmul(pS[0:mw], wb_sb[0:kw, j, 0:mw],
                                         gsA[0:kw, hs, j],
                                         start=True, stop=True)
                        dst = (GXS if j == 0 else GYS)
                        nc.scalar.copy(dst[0:PC, w, hs], pS[0:PC])
                    nc.gpsimd.tensor_tensor(mag[0:mwE, hs], sq[0:mwE, hs, 0],
                                            sq[0:mwE, hs, 1], AluOpType.add)
                    nc.scalar.sqrt(mag[0:mwE, hs], mag[0:mwE, hs])
                    nc.gpsimd.tensor_tensor(tg[0:mwE, hs], mag[0:mwE, hs, 0],
                                            mag[0:mwE, hs, 1], AluOpType.add)
                    nc.gpsimd.tensor_tensor(G[0:mwE, w, hs], tg[0:mwE, hs],
                                            mag[0:mwE, hs, 2], AluOpType.add)
                    nc.sync.dma_start(Gp1[0:PC, w, hs], G[1:PE1, w, hs])

        def nms_chunk(w):
            nms_front(w)
            Gw = G[0:PC, w, :, 1:1 + WIN]
            nc.sync.dma_start(cball[1:PC, 4, :, 1:67],
                              cball[0:PC - 1, 0, :, 1:67])
            nc.sync.dma_start(cball[1:PC, 5, :, 1:67],
                              cball[0:PC - 1, 1, :, 0:66])
            nc.sync.dma_start(cball[1:PC, 3, :, 1:67],
                              cball[0:PC - 1, 7, :, 2:68])
            # F: direct pairs multiply; derived pairs NOT a AND NOT b
            nc.vector.tensor_tensor(F[0:PC, w, 0:3],
                                    cball[0:PC, 0:3, 0:4],
                                    cball[0:PC, 0:3, 4:8],
                                    AluOpType.mult)
            nc.vector.tensor_tensor(F[0:PC, w, 7:8],
                                    cball[0:PC, 7:8, 0:4],
                                    cball[0:PC, 7:8, 4:8],
                                    AluOpType.mult)
            nc.vector.tensor_tensor(ftmp[0:PC, 0:3],
                                    cball[0:PC, 3:6, 0:4],
                                    cball[0:PC, 3:6, 4:8],
                                    AluOpType.add)
            nc.vector.tensor_tensor(ftmp[0:PC, 3:4, :, 1:68],
                                    cball[0:PC, 2:3, 0:4, 0:67],
                                    cball[0:PC, 2:3, 4:8, 0:67],
                                    AluOpType.add)
            nc.vector.tensor_single_scalar(F[0:PC, w, 3:7], ftmp[0:PC],
                                           0.5, AluOpType.is_lt)
            # 4-way select by sector class
            sel = ev.tile([128, B, GR], bf16, tag="sel", bufs=1)
            nc.vector.tensor_copy(sel[0:PC], F[0:PC, w, :, 3])
            nc.vector.copy_predicated(sel[0:PC], qsm[0:PC, w],
                                      F[0:PC, w, :, 1])
            nc.vector.copy_predicated(sel[0:PC], c1m[0:PC, w],
                                      F[0:PC, w, :, 0])
            nc.vector.copy_predicated(sel[0:PC], c2m[0:PC, w],
                                      F[0:PC, w, :, 2])
            # hysteresis masks
            nc.vector.tensor_single_scalar(him[0:PC, w], G[0:PC, w], HIGH,
                                           AluOpType.is_gt)
            nc.vector.scalar_tensor_tensor(mid[0:PC, w], G[0:PC, w], LOW,
                                           him[0:PC, w],
                                           AluOpType.is_ge, AluOpType.is_gt)
            nc.vector.tensor_tensor(mid[0:PC, w], mid[0:PC, w], sel[0:PC],
                                    AluOpType.mult)
            nc.vector.tensor_tensor(hi[0:PC, w], sel[0:PC], him[0:PC, w],
                                    AluOpType.mult)
            nc.vector.tensor_tensor(rsc[0:PC, w], hi[0:PC, w, :, 1:1 + RB],
                                    hi[0:PC, w, :, 3:3 + RB], AluOpType.add)
            nc.vector.tensor_tensor(rs2[0:PC, w], rsc[0:PC, w],
                                    hi[0:PC, w, :, 2:2 + RB], AluOpType.add)

        rsp = nm.tile([128, NW, B, RB], bf16, tag="rsp")
        rsm = nm.tile([128, NW, B, RB], bf16, tag="rsm")

        def fin_chunk(w):
            nc.sync.dma_start(rsp[0:PC - 1, w], rs2[1:PC, w])
            if w + 1 < NW:
                nc.sync.dma_start(rsp[PC - 1:PC, w], rs2[0:1, w + 1])
            nc.sync.dma_start(rsm[1:PC, w], rs2[0:PC - 1, w])
            if w > 0:
                nc.sync.dma_start(rsm[0:1, w], rs2[PC - 1:PC, w - 1])
            # s8 = 8-neighbor sum of hi; om = cond*mid = min(mid, s8);
            # out = max(om, hi)
            nc.vector.tensor_tensor(rsp[0:PC, w], rsp[0:PC, w],
                                    rsm[0:PC, w], AluOpType.add)
            nc.vector.tensor_tensor(rsp[0:PC, w], rsp[0:PC, w],
                                    rsc[0:PC, w], AluOpType.add)
            nc.vector.tensor_tensor(rsp[0:PC, w], rsp[0:PC, w],
                                    mid[0:PC, w, :, 2:2 + RB], AluOpType.min)
            nc.vector.tensor_tensor(rsp[0:PC, w], rsp[0:PC, w],
                                    hi[0:PC, w, :, 2:2 + RB], AluOpType.max)
            nc.sync.dma_start(o_d[:, w], rsp[0:PC, w])

        # interleave: conv(w), nms(w), fin(w-1)
        for w in range(NW):
            conv_chunk(w)
            nms_chunk(w)
            if w >= 1:
                fin_chunk(w - 1)
        fin_chunk(NW - 1)
    return nc


def _prep_weights(gauss_h):
    g = np.asarray(gauss_h, np.float64).reshape(-1)
    wa = np.stack([_band(np.convolve(g, [1., 2., 1.]), XR, 68),
                   _band(np.convolve(g, [1., 0., -1.]), XR, 68)], axis=1)
    wb = np.stack([_band(np.convolve(g, [1., 0., -1.]), 128, PE1),
                   _band(np.convolve(g, [1., 2., 1.]), 128, PE1)], axis=1)
    return np.ascontiguousarray(wa, np.float32), np.ascontiguousarray(wb, np.float32)


def kernel(img, gauss_h, gauss_v, sobel_h, sobel_v, directional, connect):
    import ml_dtypes
    bf = ml_dtypes.bfloat16
    img = np.asarray(img, np.float32)
    wa, wb = _prep_weights(gauss_h)
    wa_hi = wa.astype(bf)
    wa_lo = (wa - wa_hi.astype(np.float32)).astype(bf)

    if "nc" not in _CACHE:
        nc = _build()
        _split_excess_waits(nc)
        _CACHE["nc"] = nc
    nc = _CACHE["nc"]

    xp = np.zeros((B, C, H + 14, NW * CW + 11), np.float32)
    xp[:, :, 7:7 + H, 7:7 + W] = img
    in_maps = []
    for c in range(NCORES):
        r0 = RB * c
        slab = xp[:, :, r0:r0 + XR, :].reshape(B * C, XR, -1)
        chunks = np.stack([slab[:, :, CW * w:CW * w + 128] for w in range(NW)],
                          axis=0)                       # [NW, B*C, XR, 128]
        xin = np.ascontiguousarray(chunks.transpose(2, 0, 1, 3))
        x_hi = xin.astype(bf)
        x_lo = (xin - x_hi.astype(np.float32)).astype(bf)
        in_maps.append({"xh": x_hi, "xl": x_lo, "wah": wa_hi, "wal": wa_lo,
                        "wb": wb})

    global LAST_EXEC_NS
    if TRACE:
        res = run_bass_kernel_spmd(nc, in_maps, core_ids=list(range(NCORES)),
                                   trace=True)
        LAST_EXEC_NS = res.exec_time_ns
    else:
        res = run_bass_kernel_spmd(nc, in_maps, core_ids=list(range(NCORES)))

    out = np.zeros((B, 1, H, W), np.float32)
    for c in range(NCORES):
        o = np.asarray(res.results[c]["o"], np.float32)  # [PC, NW, B, RB]
        r0 = RB * c
        for w in range(NW):
            _, _, mw = _chunk_dims(w)
            p_lo = 2 if w == 0 else 0
            f_lo = CW * w + p_lo - 2
            f_hi = min(W, CW * w + mw - 2)
            n = f_hi - f_lo
            if n <= 0:
                continue
            out[:, 0, r0:r0 + RB, f_lo:f_hi] = np.transpose(
                o[p_lo:p_lo + n, w], (1, 2, 0))
    out[:, :, 0, :] = 0.0
    out[:, :, -1, :] = 0.0
    out[:, :, :, 0] = 0.0
    out[:, :, :, -1] = 0.0
    return out


def _split_excess_waits(nc, max_waits=1):
    """This walrus build allows one sync-wait per instruction; move excess
    waits onto preceding same-engine sequencer NoOps (queues are in-order)."""
    ctr = 0
    for f in nc.m.functions:
        for blk in f.blocks:
            out = []
            for inst in blk.instructions:
                si = inst.sync_info
                if si is not None and len(si.on_wait) > max_waits:
                    waits = list(si.on_wait)
                    excess, keep = waits[:-max_waits], waits[-max_waits:]
                    for i in range(0, len(excess), max_waits):
                        ctr += 1
                        nop = mybir.InstNoOp(name=f"waitfix-{ctr}", ins=[], outs=[])
                        nop.engine = inst.engine
                        nop.sync_info = mybir.SyncInfo(
                            on_wait=excess[i:i + max_waits], on_update=[])
                        out.append(nop)
                    inst.sync_info = mybir.SyncInfo(
                        on_wait=keep, on_update=list(si.on_update))
                    out.append(inst)
                else:
                    out.append(inst)
            blk.instructions = out
    return ctr
